# revision 25
# baseline (speedup 1.0000x reference)
"""Trainium2 Bass kernel for nn_CNN_pre_LSTM (dense_cnn).

Reference computation per sample (L=24):
    h = relu(conv1d(x, w11, b11))    # 1 -> 8 ch, k=3, same pad
    h = relu(conv1d(h, w12, b12))    # 8 -> 8
    h = maxpool2(h)                  # L 24 -> 12
    h = relu(conv1d(h, w21, b21))    # 8 -> 16
    h = relu(conv1d(h, w22, b22))    # 16 -> 16
    h = maxpool2(h)                  # L 12 -> 6
    y = h.reshape(96) @ Wl.T + bl    # 96 -> 24

Mapping: pure data parallel over the fused (S*B) batch across 8 cores;
16384 samples per core. On chip, activations live as [feature, batch_tile]
(features on SBUF partitions, batch on the free dim); each conv layer is
ONE dense banded matmul per 128-row output block (host-built matrices,
l-major/c-minor feature order, halo-overlapped l-halves so there is no
PSUM accumulation anywhere):

  - conv11 blocks evacuate PSUM via ACT (relu + per-partition bias);
    conv21 blocks via DVE tensor_scalar ((x+b) max 0) to balance engines.
  - pooled layers (conv12, conv22) emit parity-grouped blocks (even l at
    rows 0:48, odd l at rows 64:112 of one tensor). After a bias+relu
    evacuation, a small SBUF->SBUF DMA aligns the odd block's partitions
    and DVE tensor_max pools (all compute-engine operands must live on
    identical partition ranges; only DMA can move data across partitions).
  - every DMA is issued by the engine that produces its source (moves on
    ACT, which just computed the source; output store on GPSIMD; input
    prefetch alone on the sync queue) so no sequencer's program order
    serializes consecutive tiles.

The PE on this part runs at a fixed 1.2 GHz (the HAM clock gate never
opens even under 6us+ of continuous matmuls), so matmul cost is N/1.2GHz
per instruction and the matmul-instruction count (18 per 1024-sample
tile) is the kernel's hard floor.

The input is pre-transposed/chunked on the host to [n_tiles, 24, NT] per
core (DRAM partition strides must stay <= 32KB; 64KB strides crash the
device), and the output is produced as [n_tiles, 24, NT] fp32 and
reassembled on the host. All weights/biases ship as two packed blobs
(one DMA each at kernel start).
"""

import numpy as np

import concourse.bass as bass
import concourse.tile as tile
import concourse.mybir as mybir
from concourse import bacc
from concourse.bass_utils import run_bass_kernel_spmd

# ---------------------------------------------------------------- config
N_CORES = 8
S, B, L = 512, 256, 24
SB = S * B
CORE_N = SB // N_CORES  # 16384

# compute dtype for matmul operands / intermediate activations:
#   "fp16"  : float16 operands, fp32 PSUM accumulate, NT=1024
#   "fp32r" : fp32 bits, PE in float32r mode, NT=512
#   "fp32"  : exact fp32 (PE 4x slower), NT=512
COMPUTE = "fp16"


def _cfg(compute):
    if compute == "fp16":
        return dict(dt=mybir.dt.float16, np_dt=np.float16, nt=1024, mm_cast=None)
    if compute == "fp32r":
        return dict(
            dt=mybir.dt.float32, np_dt=np.float32, nt=512, mm_cast=mybir.dt.float32r
        )
    if compute == "fp32":
        return dict(dt=mybir.dt.float32, np_dt=np.float32, nt=512, mm_cast=None)
    raise ValueError(compute)


# ------------------------------------------------- host weight transforms
#
# Feature row orderings (all l-major, c-minor):
#   h1 block A: rows (l, c)  l in [0,13), c in [0,8)   -> 104 rows
#   h1 block B: rows (l, c)  l in [11,24)              -> 104 rows
#   conv12 out (parity): rows par*64 + lp*8 + c        -> 112 rows used
#   pooled h2:  rows [lp 0..5 x8ch | 16 pad | lp 6..11 x8ch] = 112
#   h3 block A: rows (l, c16) l in [0,7)               -> 112 rows
#   h3 block B: rows (l-5, c16) l in [5,12)            -> 112 rows
#   conv22 out (parity): rows par*64 + lp*16 + c       -> 112 rows used
#   pooled h4:  rows [lp 0..2 x16ch | 16 pad | lp 3..5 x16ch] = 112
#   out: rows j in [0,24)

def _band_first(w, l_ins, l_outs, cin, cout):
    """Dense banded matrix [len(l_ins)*cin, len(l_outs)*cout] for a k=3
    'same' conv, rows (l_in, ci) l-major, cols (l_out, co) l-major."""
    K = len(l_ins) * cin
    M = len(l_outs) * cout
    W = np.zeros((K, M), np.float32)
    for ki, li in enumerate(l_ins):
        for ci in range(cin):
            for mo, lo in enumerate(l_outs):
                d = li - lo + 1
                if 0 <= d < 3:
                    for co in range(cout):
                        W[ki * cin + ci, mo * cout + co] = w[co, ci, d]
    return W


def _band_parity(w, l_ins, l_out_base, half_l, cin, cout):
    """Banded matrix with parity-grouped output: cols = par*64 + lp*cout +
    co, l_out = l_out_base + 2*lp + par (even block cols 0:48, odd block
    cols 64:112; pads 48:64 and 112:128 are zeroed by the matmul so the
    full [128] tensor is initialized)."""
    K = len(l_ins) * cin
    W = np.zeros((K, 128), np.float32)
    for ki, li in enumerate(l_ins):
        for ci in range(cin):
            for par in range(2):
                for lp in range(half_l):
                    lo = l_out_base + 2 * lp + par
                    d = li - lo + 1
                    if 0 <= d < 3:
                        for co in range(cout):
                            W[ki * cin + ci, par * 64 + lp * cout + co] = w[co, ci, d]
    return W


def _pad48(W):
    """Insert 16 zero rows at row 48 (pooled tensors carry a pad block)."""
    return np.concatenate([W[:48], np.zeros((16,) + W.shape[1:], W.dtype), W[48:]], 0)


def _host_weights(w11, b11, w12, b12, w21, b21, w22, b22, Wl, bl):
    f32 = np.float32
    w11, w12, w21, w22, Wl = (np.asarray(a, f32) for a in (w11, w12, w21, w22, Wl))

    W11A = _band_first(w11, range(0, 24), range(0, 13), 1, 8)      # [24, 104]
    W11B = _band_first(w11, range(0, 24), range(11, 24), 1, 8)     # [24, 104]
    W12A = _band_parity(w12, range(0, 13), 0, 6, 8, 8)             # [104, 128]
    W12B = _band_parity(w12, range(11, 24), 12, 6, 8, 8)           # [104, 128]
    W21A = _pad48(_band_first(w21, range(0, 12), range(0, 7), 8, 16))   # [112, 112]
    W21B = _pad48(_band_first(w21, range(0, 12), range(5, 12), 8, 16))  # [112, 112]
    W22A = _band_parity(w22, range(0, 7), 0, 3, 16, 16)            # [112, 128]
    W22B = _band_parity(w22, range(5, 12), 6, 3, 16, 16)           # [112, 128]
    # torch flatten feature = c*6 + lp ; h4 row = lp*16 + c (plus pad48)
    WLIN = np.zeros((96, 24), f32)
    for lp in range(6):
        for c in range(16):
            WLIN[lp * 16 + c, :] = Wl[:, c * 6 + lp]
    WLIN = _pad48(WLIN)                                            # [112, 24]

    return {
        "w11a": W11A, "w11b": W11B, "w12a": W12A, "w12b": W12B,
        "w21a": W21A, "w21b": W21B, "w22a": W22A, "w22b": W22B,
        "wlin": WLIN,
        "b11v": np.tile(np.asarray(b11, f32), 13).reshape(104, 1),
        "b12v": np.tile(np.asarray(b12, f32), 16).reshape(128, 1),
        "b21v": np.tile(np.asarray(b21, f32), 7).reshape(112, 1),
        "b22v": np.tile(np.asarray(b22, f32), 8).reshape(128, 1),
        "blv": np.asarray(bl, f32).reshape(24, 1),
    }


# weight blob layout: (name, K, M) in packing order
_WSPEC = [
    ("w11a", 24, 104), ("w11b", 24, 104),
    ("w12a", 104, 128), ("w12b", 104, 128),
    ("w21a", 112, 112), ("w21b", 112, 112),
    ("w22a", 112, 128), ("w22b", 112, 128),
    ("wlin", 112, 24),
]
_WOFF = {}
_off = 0
for _n, _k, _m in _WSPEC:
    _WOFF[_n] = _off
    _off += _m
WBLOB_COLS = _off

_BSPEC = [("b11v", 104), ("b12v", 128), ("b21v", 112), ("b22v", 128), ("blv", 24)]
_BOFF = {n: i for i, (n, _) in enumerate(_BSPEC)}


def _pack_blobs(W, np_dt):
    wb = np.zeros((128, WBLOB_COLS), np_dt)
    for n, k, m in _WSPEC:
        assert W[n].shape == (k, m), (n, W[n].shape)
        wb[:k, _WOFF[n]:_WOFF[n] + m] = W[n].astype(np_dt)
    bb = np.zeros((128, len(_BSPEC)), np.float32)
    for n, p in _BSPEC:
        bb[:p, _BOFF[n]] = W[n][:, 0]
    return wb, bb


# ----------------------------------------------------- numpy device model
def emulate(x, np_dt=np.float16, **kw):
    """Pure-numpy emulation of the device dataflow (same banded matrices,
    same orderings, same cast points). Used to validate index math."""
    W = _host_weights(**kw)
    xt = np.ascontiguousarray(x.reshape(-1, L).T).astype(np_dt)  # [24, N]
    c = lambda a: a.astype(np_dt)

    def mm(wname, act):
        return c(W[wname]).astype(np.float32).T @ act.astype(np.float32)

    def relu_b(a, bias):
        return np.maximum(a + bias, 0.0)

    psA, psB = c(mm("w11a", xt)), c(mm("w11b", xt))
    h1a, h1b = c(relu_b(psA, W["b11v"])), c(relu_b(psB, W["b11v"]))
    psC, psD = c(mm("w12a", h1a)), c(mm("w12b", h1b))
    sA, sB = c(relu_b(psC, W["b12v"])), c(relu_b(psD, W["b12v"]))
    h2r = np.concatenate(
        [np.maximum(sA[0:64], sA[64:128]), np.maximum(sB[0:48], sB[64:112])], 0
    )
    psE, psF = c(mm("w21a", h2r)), c(mm("w21b", h2r))
    h3a, h3b = c(relu_b(psE, W["b21v"])), c(relu_b(psF, W["b21v"]))
    psG, psH = c(mm("w22a", h3a)), c(mm("w22b", h3b))
    sG, sH = c(relu_b(psG, W["b22v"])), c(relu_b(psH, W["b22v"]))
    h4r = np.concatenate(
        [np.maximum(sG[0:64], sG[64:128]), np.maximum(sH[0:48], sH[64:112])], 0
    )
    out = mm("wlin", h4r) + W["blv"]  # fp32
    return out.T.reshape(x.shape[0], x.shape[1], 24).astype(np.float32)


# --------------------------------------------------------- device builder
def build_kernel(n_samples, compute=COMPUTE, n_cores=N_CORES):
    cfg = _cfg(compute)
    DT, NT = cfg["dt"], cfg["nt"]
    MMC = cfg["mm_cast"]
    f32 = mybir.dt.float32
    n_tiles = n_samples // NT
    assert n_samples % NT == 0

    nc = bacc.Bacc(
        "TRN2",
        target_bir_lowering=False,
        debug=False,
        enable_asserts=False,
        num_devices=n_cores,
    )

    xt_d = nc.dram_tensor("xt", [n_tiles, 24, NT], DT, kind="ExternalInput").ap()
    wb_d = nc.dram_tensor("wblob", [128, WBLOB_COLS], DT, kind="ExternalInput").ap()
    bb_d = nc.dram_tensor("bblob", [128, len(_BSPEC)], f32,
                          kind="ExternalInput").ap()
    out_d = nc.dram_tensor("out", [n_tiles, 24, NT], f32, kind="ExternalOutput").ap()

    Relu = mybir.ActivationFunctionType.Relu
    Add, Max = mybir.AluOpType.add, mybir.AluOpType.max

    def mmop(ap):
        return ap.bitcast(MMC) if MMC is not None else ap

    # matmul fp32 PSUM output must stay inside one 2KB bank -> <=512 cols
    MMN = min(NT, 512)

    with tile.TileContext(nc) as tc:
        with (
            tc.tile_pool(name="consts", bufs=1) as cpool,
            tc.tile_pool(name="xin", bufs=6) as xpool,
            tc.tile_pool(name="acts", bufs=3) as apool,
            tc.tile_pool(name="outs", bufs=3) as opool,
            tc.tile_pool(name="ps", bufs=3, space="PSUM") as pspool,
            tc.tile_pool(name="pslin", bufs=1, space="PSUM") as lpool,
        ):
            wsb = cpool.tile([128, WBLOB_COLS], DT, tag="wblob")
            bsb = cpool.tile([128, len(_BSPEC)], f32, tag="bblob")
            nc.sync.dma_start(wsb[:], wb_d)
            nc.sync.dma_start(bsb[:], bb_d)

            def w(name):
                k, m = next((kk, mm_) for nn, kk, mm_ in _WSPEC if nn == name)
                return mmop(wsb[0:k, _WOFF[name]:_WOFF[name] + m])

            def bias(name):
                p = next(pp for nn, pp in _BSPEC if nn == name)
                return bsb[0:p, _BOFF[name]:_BOFF[name] + 1]

            def mm(out_ps, wname, rhs_sb):
                for j in range(0, NT, MMN):
                    nc.tensor.matmul(out_ps[:, j:j + MMN], w(wname),
                                     mmop(rhs_sb[:, j:j + MMN]),
                                     start=True, stop=True)

            # ---- software-pipelined emission -------------------------
            # Engines execute their instruction streams IN ORDER, so a
            # depth-first per-tile emission serializes tiles. Emitting TWO
            # skewed super-stages (layers 1-2 of tile t interleaved with
            # layers 3-5 of tile t-1) keeps every engine's queue stocked
            # with independent work while keeping pipeline fill/drain to a
            # single step (a 5-deep skew spent ~25us draining).
            h2 = {}

            def stage_a(t):
                # conv11 + conv12 + pool1 -> h2[t]
                xt_t = xpool.tile([24, NT], DT, tag="xt")
                nc.sync.dma_start(xt_t[:], xt_d[t])
                psA = pspool.tile([104, NT], f32, tag="ps")
                psB = pspool.tile([104, NT], f32, tag="ps")
                mm(psA, "w11a", xt_t)
                mm(psB, "w11b", xt_t)
                h1a = apool.tile([104, NT], DT, tag="h1a")
                h1b = apool.tile([104, NT], DT, tag="h1b")
                nc.scalar.activation(h1a[:], psA[:], Relu, bias=bias("b11v"))
                nc.scalar.activation(h1b[:], psB[:], Relu, bias=bias("b11v"))
                psC = pspool.tile([128, NT], f32, tag="ps")
                psD = pspool.tile([128, NT], f32, tag="ps")
                mm(psC, "w12a", h1a)
                mm(psD, "w12b", h1b)
                s12a = apool.tile([128, NT], DT, tag="s12a")
                s12b = apool.tile([128, NT], DT, tag="s12b")
                nc.scalar.activation(s12a[:], psC[:], Relu, bias=bias("b12v"))
                nc.scalar.activation(s12b[:], psD[:], Relu, bias=bias("b12v"))
                mv1 = apool.tile([64, NT], DT, tag="mv1")
                mv2 = apool.tile([112, NT], DT, tag="mv2")
                nc.scalar.dma_start(mv1[0:64, :], s12a[64:128, :])
                nc.scalar.dma_start(mv2[64:112, :], s12b[0:48, :])
                h2r = apool.tile([112, NT], DT, tag="h2r")
                nc.vector.tensor_max(h2r[0:64, :], s12a[0:64, :], mv1[0:64, :])
                nc.vector.tensor_max(h2r[64:112, :], s12b[64:112, :],
                                     mv2[64:112, :])
                h2[t] = h2r

            def stage_b(t):
                # conv21 + conv22 + pool2 + linear + store
                h2r = h2.pop(t)
                psE = pspool.tile([112, NT], f32, tag="ps")
                psF = pspool.tile([112, NT], f32, tag="ps")
                mm(psE, "w21a", h2r)
                mm(psF, "w21b", h2r)
                h3a = apool.tile([112, NT], DT, tag="h3a")
                h3b = apool.tile([112, NT], DT, tag="h3b")
                nc.vector.tensor_scalar(h3a[:], psE[:], bias("b21v"), 0.0,
                                        Add, Max)
                nc.vector.tensor_scalar(h3b[:], psF[:], bias("b21v"), 0.0,
                                        Add, Max)
                psG = pspool.tile([128, NT], f32, tag="ps")
                psH = pspool.tile([128, NT], f32, tag="ps")
                mm(psG, "w22a", h3a)
                mm(psH, "w22b", h3b)
                s22a = apool.tile([128, NT], DT, tag="s22a")
                s22b = apool.tile([128, NT], DT, tag="s22b")
                nc.scalar.activation(s22a[:], psG[:], Relu, bias=bias("b22v"))
                nc.scalar.activation(s22b[:], psH[:], Relu, bias=bias("b22v"))
                mv3 = apool.tile([64, NT], DT, tag="mv3")
                mv4 = apool.tile([112, NT], DT, tag="mv4")
                nc.scalar.dma_start(mv3[0:64, :], s22a[64:128, :])
                nc.scalar.dma_start(mv4[64:112, :], s22b[0:48, :])
                h4r = apool.tile([112, NT], DT, tag="h4r")
                nc.vector.tensor_max(h4r[0:64, :], s22a[0:64, :], mv3[0:64, :])
                nc.vector.tensor_max(h4r[64:112, :], s22b[64:112, :],
                                     mv4[64:112, :])
                psI = lpool.tile([24, NT], f32, tag="pslin")
                mm(psI, "wlin", h4r)
                osb = opool.tile([24, NT], f32, tag="osb")
                nc.vector.tensor_scalar_add(osb[:], psI[:], bias("blv"))
                nc.gpsimd.dma_start(out_d[t], osb[:])

            for step in range(n_tiles + 1):
                if step < n_tiles:
                    stage_a(step)
                if step >= 1:
                    stage_b(step - 1)

    nc.compile()
    return nc


# ------------------------------------------------------------- entry point
def _prep_in_maps(x, weights, compute=COMPUTE):
    cfg = _cfg(compute)
    np_dt = cfg["np_dt"]
    nt = cfg["nt"]
    W = _host_weights(**weights)
    wb, bb = _pack_blobs(W, np_dt)
    xt = np.ascontiguousarray(x.reshape(SB, L).T).astype(np_dt)  # [24, SB]
    in_maps = []
    for c in range(N_CORES):
        xc = xt[:, c * CORE_N:(c + 1) * CORE_N]  # [24, CORE_N]
        in_maps.append({
            "xt": np.ascontiguousarray(
                xc.reshape(24, CORE_N // nt, nt).transpose(1, 0, 2)
            ),
            "wblob": wb,
            "bblob": bb,
        })
    return in_maps


def kernel(x, w11, b11, w12, b12, w21, b21, w22, b22, Wl, bl):
    weights = dict(w11=w11, b11=b11, w12=w12, b12=b12, w21=w21, b21=b21,
                   w22=w22, b22=b22, Wl=Wl, bl=bl)
    x = np.asarray(x, np.float32)
    nc = build_kernel(CORE_N, COMPUTE)
    in_maps = _prep_in_maps(x, weights, COMPUTE)
    res = run_bass_kernel_spmd(nc, in_maps, list(range(N_CORES)))
    outs = [
        res.results[c]["out"].transpose(1, 0, 2).reshape(24, CORE_N)
        for c in range(N_CORES)
    ]
    full = np.concatenate(outs, axis=1)  # [24, SB]
    return np.ascontiguousarray(full.T).reshape(S, B, 24).astype(np.float32)


# revision 27
# speedup vs baseline: 1.3058x; 1.3058x over previous
"""Trainium2 Bass kernel for nn_CNN_pre_LSTM (dense_cnn).

Reference computation per sample (L=24):
    h = relu(conv1d(x, w11, b11))    # 1 -> 8 ch, k=3, same pad
    h = relu(conv1d(h, w12, b12))    # 8 -> 8
    h = maxpool2(h)                  # L 24 -> 12
    h = relu(conv1d(h, w21, b21))    # 8 -> 16
    h = relu(conv1d(h, w22, b22))    # 16 -> 16
    h = maxpool2(h)                  # L 12 -> 6
    y = h.reshape(96) @ Wl.T + bl    # 96 -> 24

Mapping: pure data parallel over the fused (S*B) batch across 8 cores;
16384 samples per core. On chip, activations live as [feature, batch_tile]
(features on SBUF partitions, batch on the free dim); each conv layer is
ONE dense banded matmul per 128-row output block (host-built matrices,
l-major/c-minor feature order, halo-overlapped l-halves so there is no
PSUM accumulation anywhere):

  - conv11 blocks evacuate PSUM via ACT (relu + per-partition bias);
    conv21 blocks via DVE tensor_scalar ((x+b) max 0) to balance engines.
  - pooled layers (conv12, conv22) emit parity-grouped blocks (even l at
    rows 0:48, odd l at rows 64:112 of one tensor). After a bias+relu
    evacuation, a small SBUF->SBUF DMA aligns the odd block's partitions
    and DVE tensor_max pools (all compute-engine operands must live on
    identical partition ranges; only DMA can move data across partitions).
  - every DMA is issued by the engine that produces its source (moves on
    ACT, which just computed the source; output store on GPSIMD; input
    prefetch alone on the sync queue) so no sequencer's program order
    serializes consecutive tiles.

The PE on this part runs at a fixed 1.2 GHz (the HAM clock gate never
opens even under 6us+ of continuous matmuls), so matmul cost is N/1.2GHz
per instruction and the matmul-instruction count (18 per 1024-sample
tile) is the kernel's hard floor.

The input is pre-transposed/chunked on the host to [n_tiles, 24, NT] per
core (DRAM partition strides must stay <= 32KB; 64KB strides crash the
device), and the output is produced as [n_tiles, 24, NT] fp32 and
reassembled on the host. All weights/biases ship as two packed blobs
(one DMA each at kernel start).
"""

import numpy as np

import concourse.bass as bass
import concourse.tile as tile
import concourse.mybir as mybir
from concourse import bacc
from concourse.bass_utils import run_bass_kernel_spmd

# ---------------------------------------------------------------- config
N_CORES = 8
S, B, L = 512, 256, 24
SB = S * B
CORE_N = SB // N_CORES  # 16384

# compute dtype for matmul operands / intermediate activations:
#   "fp16"  : float16 operands, fp32 PSUM accumulate, NT=1024
#   "fp32r" : fp32 bits, PE in float32r mode, NT=512
#   "fp32"  : exact fp32 (PE 4x slower), NT=512
COMPUTE = "fp16"


def _cfg(compute):
    if compute == "fp16":
        return dict(dt=mybir.dt.float16, np_dt=np.float16, nt=1024, mm_cast=None)
    if compute == "fp32r":
        return dict(
            dt=mybir.dt.float32, np_dt=np.float32, nt=512, mm_cast=mybir.dt.float32r
        )
    if compute == "fp32":
        return dict(dt=mybir.dt.float32, np_dt=np.float32, nt=512, mm_cast=None)
    raise ValueError(compute)


# ------------------------------------------------- host weight transforms
#
# Feature row orderings (all l-major, c-minor):
#   h1 block A: rows (l, c)  l in [0,13), c in [0,8)   -> 104 rows
#   h1 block B: rows (l, c)  l in [11,24)              -> 104 rows
#   conv12 out (parity): rows par*64 + lp*8 + c        -> 112 rows used
#   pooled h2:  rows [lp 0..5 x8ch | 16 pad | lp 6..11 x8ch] = 112
#   h3 block A: rows (l, c16) l in [0,7)               -> 112 rows
#   h3 block B: rows (l-5, c16) l in [5,12)            -> 112 rows
#   conv22 out (parity): rows par*64 + lp*16 + c       -> 112 rows used
#   pooled h4:  rows [lp 0..2 x16ch | 16 pad | lp 3..5 x16ch] = 112
#   out: rows j in [0,24)

def _band_first(w, l_ins, l_outs, cin, cout):
    """Dense banded matrix [len(l_ins)*cin, len(l_outs)*cout] for a k=3
    'same' conv, rows (l_in, ci) l-major, cols (l_out, co) l-major."""
    K = len(l_ins) * cin
    M = len(l_outs) * cout
    W = np.zeros((K, M), np.float32)
    for ki, li in enumerate(l_ins):
        for ci in range(cin):
            for mo, lo in enumerate(l_outs):
                d = li - lo + 1
                if 0 <= d < 3:
                    for co in range(cout):
                        W[ki * cin + ci, mo * cout + co] = w[co, ci, d]
    return W


def _band_parity(w, l_ins, l_out_base, half_l, cin, cout):
    """Banded matrix with parity-grouped output: cols = par*64 + lp*cout +
    co, l_out = l_out_base + 2*lp + par (even block cols 0:48, odd block
    cols 64:112; pads 48:64 and 112:128 are zeroed by the matmul so the
    full [128] tensor is initialized)."""
    K = len(l_ins) * cin
    W = np.zeros((K, 128), np.float32)
    for ki, li in enumerate(l_ins):
        for ci in range(cin):
            for par in range(2):
                for lp in range(half_l):
                    lo = l_out_base + 2 * lp + par
                    d = li - lo + 1
                    if 0 <= d < 3:
                        for co in range(cout):
                            W[ki * cin + ci, par * 64 + lp * cout + co] = w[co, ci, d]
    return W


def _pad48(W):
    """Insert 16 zero rows at row 48 (pooled tensors carry a pad block)."""
    return np.concatenate([W[:48], np.zeros((16,) + W.shape[1:], W.dtype), W[48:]], 0)


def _host_weights(w11, b11, w12, b12, w21, b21, w22, b22, Wl, bl):
    f32 = np.float32
    w11, w12, w21, w22, Wl = (np.asarray(a, f32) for a in (w11, w12, w21, w22, Wl))

    W11A = _band_first(w11, range(0, 24), range(0, 13), 1, 8)      # [24, 104]
    W11B = _band_first(w11, range(0, 24), range(11, 24), 1, 8)     # [24, 104]
    W12A = _band_parity(w12, range(0, 13), 0, 6, 8, 8)             # [104, 128]
    W12B = _band_parity(w12, range(11, 24), 12, 6, 8, 8)           # [104, 128]
    W21A = _pad48(_band_first(w21, range(0, 12), range(0, 7), 8, 16))   # [112, 112]
    W21B = _pad48(_band_first(w21, range(0, 12), range(5, 12), 8, 16))  # [112, 112]
    W22A = _band_parity(w22, range(0, 7), 0, 3, 16, 16)            # [112, 128]
    W22B = _band_parity(w22, range(5, 12), 6, 3, 16, 16)           # [112, 128]
    # torch flatten feature = c*6 + lp ; h4 row = lp*16 + c (plus pad48)
    WLIN = np.zeros((96, 24), f32)
    for lp in range(6):
        for c in range(16):
            WLIN[lp * 16 + c, :] = Wl[:, c * 6 + lp]
    WLIN = _pad48(WLIN)                                            # [112, 24]

    return {
        "w11a": W11A, "w11b": W11B, "w12a": W12A, "w12b": W12B,
        "w21a": W21A, "w21b": W21B, "w22a": W22A, "w22b": W22B,
        "wlin": WLIN,
        "b11v": np.tile(np.asarray(b11, f32), 13).reshape(104, 1),
        "b12v": np.tile(np.asarray(b12, f32), 16).reshape(128, 1),
        "b21v": np.tile(np.asarray(b21, f32), 7).reshape(112, 1),
        "b22v": np.tile(np.asarray(b22, f32), 8).reshape(128, 1),
        "blv": np.asarray(bl, f32).reshape(24, 1),
    }


# weight blob layout: (name, K, M) in packing order
_WSPEC = [
    ("w11a", 24, 104), ("w11b", 24, 104),
    ("w12a", 104, 128), ("w12b", 104, 128),
    ("w21a", 112, 112), ("w21b", 112, 112),
    ("w22a", 112, 128), ("w22b", 112, 128),
    ("wlin", 112, 24),
]
_WOFF = {}
_off = 0
for _n, _k, _m in _WSPEC:
    _WOFF[_n] = _off
    _off += _m
WBLOB_COLS = _off

_BSPEC = [("b11v", 104), ("b12v", 128), ("b21v", 112), ("b22v", 128), ("blv", 24)]
_BOFF = {n: i for i, (n, _) in enumerate(_BSPEC)}


def _pack_blobs(W, np_dt):
    wb = np.zeros((128, WBLOB_COLS), np_dt)
    for n, k, m in _WSPEC:
        assert W[n].shape == (k, m), (n, W[n].shape)
        wb[:k, _WOFF[n]:_WOFF[n] + m] = W[n].astype(np_dt)
    bb = np.zeros((128, len(_BSPEC)), np.float32)
    for n, p in _BSPEC:
        bb[:p, _BOFF[n]] = W[n][:, 0]
    return wb, bb


# ----------------------------------------------------- numpy device model
def emulate(x, np_dt=np.float16, **kw):
    """Pure-numpy emulation of the device dataflow (same banded matrices,
    same orderings, same cast points). Used to validate index math."""
    W = _host_weights(**kw)
    xt = np.ascontiguousarray(x.reshape(-1, L).T).astype(np_dt)  # [24, N]
    c = lambda a: a.astype(np_dt)

    def mm(wname, act):
        return c(W[wname]).astype(np.float32).T @ act.astype(np.float32)

    def relu_b(a, bias):
        return np.maximum(a + bias, 0.0)

    psA, psB = c(mm("w11a", xt)), c(mm("w11b", xt))
    h1a, h1b = c(relu_b(psA, W["b11v"])), c(relu_b(psB, W["b11v"]))
    psC, psD = c(mm("w12a", h1a)), c(mm("w12b", h1b))
    sA, sB = c(relu_b(psC, W["b12v"])), c(relu_b(psD, W["b12v"]))
    h2r = np.concatenate(
        [np.maximum(sA[0:64], sA[64:128]), np.maximum(sB[0:48], sB[64:112])], 0
    )
    psE, psF = c(mm("w21a", h2r)), c(mm("w21b", h2r))
    h3a, h3b = c(relu_b(psE, W["b21v"])), c(relu_b(psF, W["b21v"]))
    psG, psH = c(mm("w22a", h3a)), c(mm("w22b", h3b))
    sG, sH = c(relu_b(psG, W["b22v"])), c(relu_b(psH, W["b22v"]))
    h4r = np.concatenate(
        [np.maximum(sG[0:64], sG[64:128]), np.maximum(sH[0:48], sH[64:112])], 0
    )
    out = mm("wlin", h4r) + W["blv"]  # fp32
    return out.T.reshape(x.shape[0], x.shape[1], 24).astype(np.float32)


# --------------------------------------------------------- device builder
def build_kernel(n_samples, compute=COMPUTE, n_cores=N_CORES):
    cfg = _cfg(compute)
    DT, NT = cfg["dt"], cfg["nt"]
    MMC = cfg["mm_cast"]
    f32 = mybir.dt.float32
    n_tiles = n_samples // NT
    assert n_samples % NT == 0

    nc = bacc.Bacc(
        "TRN2",
        target_bir_lowering=False,
        debug=False,
        enable_asserts=False,
        num_devices=n_cores,
    )

    xt_d = nc.dram_tensor("xt", [n_tiles, 24, NT], DT, kind="ExternalInput").ap()
    wb_d = nc.dram_tensor("wblob", [128, WBLOB_COLS], DT, kind="ExternalInput").ap()
    bb_d = nc.dram_tensor("bblob", [128, len(_BSPEC)], f32,
                          kind="ExternalInput").ap()
    out_d = nc.dram_tensor("out", [n_tiles, 24, NT], f32, kind="ExternalOutput").ap()

    Relu = mybir.ActivationFunctionType.Relu
    Add, Max = mybir.AluOpType.add, mybir.AluOpType.max

    def mmop(ap):
        return ap.bitcast(MMC) if MMC is not None else ap

    # matmul fp32 PSUM output must stay inside one 2KB bank -> <=512 cols
    MMN = min(NT, 512)

    with tile.TileContext(nc) as tc:
        with (
            tc.tile_pool(name="consts", bufs=1) as cpool,
            tc.tile_pool(name="xin", bufs=6) as xpool,
            tc.tile_pool(name="acts", bufs=4) as apool,
            tc.tile_pool(name="outs", bufs=3) as opool,
            tc.tile_pool(name="ps", bufs=3, space="PSUM") as pspool,
            tc.tile_pool(name="pslin", bufs=1, space="PSUM") as lpool,
        ):
            # prefetch the ACT spline-table set (~2.7us) during the blob
            # DMAs: a dummy ACTIVATE forces walrus to place the table load
            # at the head of ACT's stream instead of before tile 0's evac
            warm = cpool.tile([1, 2], f32, tag="actwarm")
            nc.vector.memset(warm[:], 0.0)
            nc.scalar.activation(warm[:], warm[:], Relu, bias=0.0)

            wsb = cpool.tile([128, WBLOB_COLS], DT, tag="wblob")
            bsb = cpool.tile([128, len(_BSPEC)], f32, tag="bblob")
            nc.sync.dma_start(wsb[:], wb_d)
            nc.sync.dma_start(bsb[:], bb_d)

            def w(name):
                k, m = next((kk, mm_) for nn, kk, mm_ in _WSPEC if nn == name)
                return mmop(wsb[0:k, _WOFF[name]:_WOFF[name] + m])

            def bias(name):
                p = next(pp for nn, pp in _BSPEC if nn == name)
                return bsb[0:p, _BOFF[name]:_BOFF[name] + 1]

            def mm(out_ps, wname, rhs_sb):
                for j in range(0, NT, MMN):
                    nc.tensor.matmul(out_ps[:, j:j + MMN], w(wname),
                                     mmop(rhs_sb[:, j:j + MMN]),
                                     start=True, stop=True)

            # ---- software-pipelined emission -------------------------
            # Engines execute their instruction streams IN ORDER, so a
            # depth-first per-tile emission serializes tiles (the PE sits
            # behind its own next-layer matmuls, which wait on the current
            # tile's evacuations). Emitting the five stages SKEWED across
            # tiles interleaves independent work in every engine's queue.
            h1 = {}
            h2 = {}
            h3 = {}
            h4 = {}

            def s1_conv11(t):
                xt_t = xpool.tile([24, NT], DT, tag="xt")
                nc.sync.dma_start(xt_t[:], xt_d[t])
                psA = pspool.tile([104, NT], f32, tag="ps")
                psB = pspool.tile([104, NT], f32, tag="ps")
                mm(psA, "w11a", xt_t)
                mm(psB, "w11b", xt_t)
                h1a = apool.tile([104, NT], DT, tag="h1a")
                h1b = apool.tile([104, NT], DT, tag="h1b")
                nc.scalar.activation(h1a[:], psA[:], Relu, bias=bias("b11v"))
                nc.scalar.activation(h1b[:], psB[:], Relu, bias=bias("b11v"))
                h1[t] = (h1a, h1b)

            def s2_conv12(t):
                h1a, h1b = h1.pop(t)
                psC = pspool.tile([128, NT], f32, tag="ps")
                psD = pspool.tile([128, NT], f32, tag="ps")
                mm(psC, "w12a", h1a)
                mm(psD, "w12b", h1b)
                s12a = apool.tile([128, NT], DT, tag="s12a")
                s12b = apool.tile([128, NT], DT, tag="s12b")
                nc.scalar.activation(s12a[:], psC[:], Relu, bias=bias("b12v"))
                nc.scalar.activation(s12b[:], psD[:], Relu, bias=bias("b12v"))
                mv1 = apool.tile([64, NT], DT, tag="mv1")
                mv2 = apool.tile([112, NT], DT, tag="mv2")
                nc.scalar.dma_start(mv1[0:64, :], s12a[64:128, :])
                nc.scalar.dma_start(mv2[64:112, :], s12b[0:48, :])
                h2r = apool.tile([112, NT], DT, tag="h2r")
                nc.vector.tensor_max(h2r[0:64, :], s12a[0:64, :], mv1[0:64, :])
                nc.vector.tensor_max(h2r[64:112, :], s12b[64:112, :],
                                     mv2[64:112, :])
                h2[t] = h2r

            def s3_conv21(t):
                h2r = h2.pop(t)
                psE = pspool.tile([112, NT], f32, tag="ps")
                psF = pspool.tile([112, NT], f32, tag="ps")
                mm(psE, "w21a", h2r)
                mm(psF, "w21b", h2r)
                h3a = apool.tile([112, NT], DT, tag="h3a")
                h3b = apool.tile([112, NT], DT, tag="h3b")
                nc.vector.tensor_scalar(h3a[:], psE[:], bias("b21v"), 0.0,
                                        Add, Max)
                nc.vector.tensor_scalar(h3b[:], psF[:], bias("b21v"), 0.0,
                                        Add, Max)
                h3[t] = (h3a, h3b)

            def s4_conv22(t):
                h3a, h3b = h3.pop(t)
                psG = pspool.tile([128, NT], f32, tag="ps")
                psH = pspool.tile([128, NT], f32, tag="ps")
                mm(psG, "w22a", h3a)
                mm(psH, "w22b", h3b)
                s22a = apool.tile([128, NT], DT, tag="s22a")
                s22b = apool.tile([128, NT], DT, tag="s22b")
                nc.scalar.activation(s22a[:], psG[:], Relu, bias=bias("b22v"))
                nc.scalar.activation(s22b[:], psH[:], Relu, bias=bias("b22v"))
                mv3 = apool.tile([64, NT], DT, tag="mv3")
                mv4 = apool.tile([112, NT], DT, tag="mv4")
                nc.scalar.dma_start(mv3[0:64, :], s22a[64:128, :])
                nc.scalar.dma_start(mv4[64:112, :], s22b[0:48, :])
                h4r = apool.tile([112, NT], DT, tag="h4r")
                nc.vector.tensor_max(h4r[0:64, :], s22a[0:64, :], mv3[0:64, :])
                nc.vector.tensor_max(h4r[64:112, :], s22b[64:112, :],
                                     mv4[64:112, :])
                h4[t] = h4r

            def s5_linear(t):
                h4r = h4.pop(t)
                psI = lpool.tile([24, NT], f32, tag="pslin")
                mm(psI, "wlin", h4r)
                osb = opool.tile([24, NT], f32, tag="osb")
                nc.vector.tensor_scalar_add(osb[:], psI[:], bias("blv"))
                nc.gpsimd.dma_start(out_d[t], osb[:])

            stages = [s1_conv11, s2_conv12, s3_conv21, s4_conv22, s5_linear]
            for step in range(n_tiles + len(stages) - 1):
                for s, fn in enumerate(stages):
                    t = step - s
                    if 0 <= t < n_tiles:
                        fn(t)

    nc.compile()
    return nc


# ------------------------------------------------------------- entry point
def _prep_in_maps(x, weights, compute=COMPUTE):
    cfg = _cfg(compute)
    np_dt = cfg["np_dt"]
    nt = cfg["nt"]
    W = _host_weights(**weights)
    wb, bb = _pack_blobs(W, np_dt)
    xt = np.ascontiguousarray(x.reshape(SB, L).T).astype(np_dt)  # [24, SB]
    in_maps = []
    for c in range(N_CORES):
        xc = xt[:, c * CORE_N:(c + 1) * CORE_N]  # [24, CORE_N]
        in_maps.append({
            "xt": np.ascontiguousarray(
                xc.reshape(24, CORE_N // nt, nt).transpose(1, 0, 2)
            ),
            "wblob": wb,
            "bblob": bb,
        })
    return in_maps


def kernel(x, w11, b11, w12, b12, w21, b21, w22, b22, Wl, bl):
    weights = dict(w11=w11, b11=b11, w12=w12, b12=b12, w21=w21, b21=b21,
                   w22=w22, b22=b22, Wl=Wl, bl=bl)
    x = np.asarray(x, np.float32)
    nc = build_kernel(CORE_N, COMPUTE)
    in_maps = _prep_in_maps(x, weights, COMPUTE)
    res = run_bass_kernel_spmd(nc, in_maps, list(range(N_CORES)))
    outs = [
        res.results[c]["out"].transpose(1, 0, 2).reshape(24, CORE_N)
        for c in range(N_CORES)
    ]
    full = np.concatenate(outs, axis=1)  # [24, SB]
    return np.ascontiguousarray(full.T).reshape(S, B, 24).astype(np.float32)
